# revision 3
# baseline (speedup 1.0000x reference)
"""Trainium2 Bass kernel for nn_MessagePassing (gnn_message_passing) — v3.

Reference computation (2 steps):
    h    = relu(cur @ mW1 + mb1)
    msg  = h @ mW2 + mb2
    rec  = einsum('mn,bnd->bmd', C, msg) * w[:,:,None]
    g    = relu(concat([cur, rec], -1) @ aW1 + ab1)
    cur  = cur + g @ aW2 + ab2

Strategy (data-parallel over 8 NeuronCores, 8192 batch rows each):
  * [feat=128 partitions, cols] on-chip layout; col = (b, m) of the
    flattened [B*M, 128] stream; 1024-col chunks.
  * Algebraic folds (host): Q' = mW2 @ aW1_bot replaces the mW2 pass;
    qb = mb2 @ aW1_bot is added to hq during its evacuation (a DVE
    tensor_tensor against a constant tile), which the mix turns into
    the exact w*s(m)*qb term the mb2 bias would have produced; ab2 is
    folded into step-1 biases (mb1 += mW1^T ab2, ab1 += aW1top^T ab2)
    and both step ab2's are restored on the host (out += 2*ab2).
  * NO DMA transposes and NO separate w-scale stage (the two big
    costs of the previous design). Instead, two role-swapped matmul
    passes per step (stationary = data block, moving = weights):
      q-sw : lhsT = h[:,blk],   rhs = Q'        -> hqT [col, fo] PSUM
      mix-sw: lhsT = hqT[:,blk], rhs = wCk[blk] -> g_pre [fo, col] PSUM
    where wCk[colin, colout] = kron(I16, C.T)[colin, colout] *
    w[colout] is built on the HOST per 128-col block (module_weights
    are inputs, so the w-scale rides the moving operand for free).
    mix-sw accumulates straight onto the a1t pass in PSUM.
  * Per step: m1 (2xMM512) -> ACT relu+mb1 -> q-sw (8 LDW+MM128
    pairs) -> DVE +qb evac -> a1t + mix-sw (accumulated in one PSUM;
    a1t first: start=True clears the whole bank) -> ACT relu+ab1 ->
    a2 (2xMM512) -> DVE residual add. Stages staggered so every
    cross-engine dependency is >=1 tick old; 4 PSUM tags x 2 banks.
    Measured: LDW+MM(N=128) pairs stream at 56 ns warm (LDWEIGHTS
    fully hidden), so swapped passes cost ~1 cyc/col like normal ones.
"""

import sys

import numpy as np

try:
    import concourse.bass as bass  # noqa: F401
except ImportError:  # harness runs kernel.py from a bare directory
    sys.path.insert(0, "/opt/trn_rl_repo")
    import concourse.bass as bass  # noqa: F401

import ml_dtypes
import concourse.bacc as bacc
import concourse.mybir as mybir
from concourse.tile import TileContext

BF16 = ml_dtypes.bfloat16
D = 128
M = 8
CHUNK = 1024
NBLK = CHUNK // D       # 8
GRP = 512               # matmul moving free dim (1 fp32 PSUM bank)
NCORES = 8
W_LAG = 3               # step-1 stream lag in chunks
PF = 4                  # prefetch distance in ticks

_nc_cache = {}


def build_nc(cols):
    if cols in _nc_cache:
        return _nc_cache[cols]
    assert cols % CHUNK == 0
    nchunk = cols // CHUNK

    f32 = mybir.dt.float32
    bf = mybir.dt.bfloat16
    relu = mybir.ActivationFunctionType.Relu
    copyf = mybir.ActivationFunctionType.Copy
    add = mybir.AluOpType.add
    amax = mybir.AluOpType.max

    nc = bacc.Bacc(trn_type="TRN2")
    xb_d = nc.declare_dram_parameter("xb", [nchunk, D, CHUNK], bf, isOutput=False)
    wck_d = nc.declare_dram_parameter("wck", [nchunk, D, CHUNK], bf, isOutput=False)
    wm1_d = nc.declare_dram_parameter("wm1", [D, D], bf, isOutput=False)
    wq_d = nc.declare_dram_parameter("wq", [D, D], bf, isOutput=False)
    wa1t_d = nc.declare_dram_parameter("wa1t", [D, D], bf, isOutput=False)
    wa2_d = nc.declare_dram_parameter("wa2", [D, D], bf, isOutput=False)
    mb10_d = nc.declare_dram_parameter("mb10", [D, 1], f32, isOutput=False)
    mb11_d = nc.declare_dram_parameter("mb11", [D, 1], f32, isOutput=False)
    qbt_d = nc.declare_dram_parameter("qbt", [D, CHUNK], f32, isOutput=False)
    ab10_d = nc.declare_dram_parameter("ab10", [D, 1], f32, isOutput=False)
    ab11_d = nc.declare_dram_parameter("ab11", [D, 1], f32, isOutput=False)
    out_d = nc.declare_dram_parameter("out", [nchunk, D, CHUNK], bf, isOutput=True)

    with TileContext(nc) as tc:
        with (
            tc.tile_pool(name="consts", bufs=1) as cp,
            tc.tile_pool(name="work", bufs=2) as wp,
            tc.tile_pool(name="pipe", bufs=3) as fp,
            tc.tile_pool(name="psum", bufs=1, space="PSUM") as pp,
        ):
            pool = mybir.EngineType.SP
            w_m1 = cp.tile_from(forced_dma_engine=pool, ap=wm1_d[:, :])
            w_q = cp.tile_from(forced_dma_engine=pool, ap=wq_d[:, :])
            w_a1t = cp.tile_from(forced_dma_engine=pool, ap=wa1t_d[:, :])
            w_a2 = cp.tile_from(forced_dma_engine=pool, ap=wa2_d[:, :])
            mb1s = [
                cp.tile_from(forced_dma_engine=pool, ap=mb10_d[:, :],
                             name="mb10"),
                cp.tile_from(forced_dma_engine=pool, ap=mb11_d[:, :],
                             name="mb11"),
            ]
            qbt = cp.tile_from(forced_dma_engine=pool, ap=qbt_d[:, :])
            ab1s = [
                cp.tile_from(forced_dma_engine=pool, ap=ab10_d[:, :],
                             name="ab10"),
                cp.tile_from(forced_dma_engine=pool, ap=ab11_d[:, :],
                             name="ab11"),
            ]

            xb_t = {}    # c -> input tile (step-0 operand + residual base)
            wck_t = {}   # c -> w-scaled kron(C) moving blocks
            c1b_t = {}   # c -> step-1 operand (x + up0, biases folded)
            h_t = {}     # (c,s) -> relu'd h
            hqs_t = {}   # (c,s) -> hqT in SBUF
            gt_t = {}    # (c,s) -> relu'd gate
            hp_ps = {}   # live psum tiles per (c,s)
            hq_ps = {}
            gp_ps = {}
            up_ps = {}

            def load(c):
                xb_t[c] = fp.tile([D, CHUNK], bf, tag="xb", bufs=6, name=f"xb{c}")
                nc.gpsimd.dma_start(xb_t[c][:], xb_d[c])
                wck_t[c] = fp.tile([D, CHUNK], bf, tag="wck", bufs=9, name=f"wck{c}")
                nc.sync.dma_start(wck_t[c][:], wck_d[c])

            def cur_of(c, s):
                return xb_t[c] if s == 0 else c1b_t[c]

            def s1_m1(c, s):
                hp = pp.tile([D, CHUNK], f32, tag="hp", name="hp")
                hp_ps[(c, s)] = hp
                cur = cur_of(c, s)
                for j in range(CHUNK // GRP):
                    cs = slice(j * GRP, (j + 1) * GRP)
                    nc.tensor.matmul(hp[:, cs], w_m1[:, :], cur[:, cs],
                                     start=True, stop=True)

            def s2_relu(c, s):
                hp = hp_ps.pop((c, s))
                h = wp.tile([D, CHUNK], bf, tag="h", bufs=4, name="h")
                nc.scalar.activation(h[:], hp[:], relu, bias=mb1s[s][:])
                h_t[(c, s)] = h

            def s3_qsw(c, s):
                h = h_t.pop((c, s))
                hq = pp.tile([D, CHUNK], f32, tag="hq", name="hq")
                hq_ps[(c, s)] = hq
                for b in range(NBLK):
                    bs = slice(b * D, (b + 1) * D)
                    nc.tensor.matmul(hq[:, bs], h[:, bs], w_q[:, :],
                                     start=True, stop=True)

            def s4_copy(c, s):
                hq = hq_ps.pop((c, s))
                hqs = wp.tile([D, CHUNK], bf, tag="hqs", bufs=3, name="hqs")
                # hqs = hq + qb (the mb2 bias becomes w*s(m)*qb after mix-sw)
                nc.vector.tensor_tensor(hqs[:], hq[:], qbt[:], add)
                hqs_t[(c, s)] = hqs

            def s5_a1mix(c, s):
                hqs = hqs_t.pop((c, s))
                gp = pp.tile([D, CHUNK], f32, tag="gp", name="gp")
                gp_ps[(c, s)] = gp
                cur = cur_of(c, s)
                wck = wck_t[c]
                for j in range(CHUNK // GRP):
                    cs = slice(j * GRP, (j + 1) * GRP)
                    nc.tensor.matmul(gp[:, cs], w_a1t[:, :], cur[:, cs],
                                     start=True, stop=False)
                for b in range(NBLK):
                    bs = slice(b * D, (b + 1) * D)
                    nc.tensor.matmul(gp[:, bs], hqs[:, bs], wck[:, bs],
                                     start=False, stop=True)

            def s6_relu(c, s):
                gp = gp_ps.pop((c, s))
                gt = wp.tile([D, CHUNK], bf, tag="gt", bufs=3, name="gt")
                nc.scalar.activation(gt[:], gp[:], relu, bias=ab1s[s][:])
                gt_t[(c, s)] = gt

            def s7_a2(c, s):
                gt = gt_t.pop((c, s))
                up = pp.tile([D, CHUNK], f32, tag="up", name="up")
                up_ps[(c, s)] = up
                for j in range(CHUNK // GRP):
                    cs = slice(j * GRP, (j + 1) * GRP)
                    nc.tensor.matmul(up[:, cs], w_a2[:, :], gt[:, cs],
                                     start=True, stop=True)

            def s8_res(c, s):
                up = up_ps.pop((c, s))
                if s == 0:
                    c1b = fp.tile([D, CHUNK], bf, tag="c1b", bufs=6,
                                  name=f"c1b{c}")
                    nc.vector.tensor_tensor(c1b[:], up[:], xb_t[c][:], add)
                    c1b_t[c] = c1b
                    del xb_t[c]
                else:
                    onew = wp.tile([D, CHUNK], bf, tag="onew", bufs=3,
                                   name="onew")
                    nc.vector.tensor_tensor(onew[:], up[:], c1b_t[c][:], add)
                    nc.gpsimd.dma_start(out_d[c], onew[:])
                    del c1b_t[c]

            def job_at(t):
                if t < 0:
                    return None
                if t % 2 == 0:
                    c = t // 2
                    return (c, 0) if c < nchunk else None
                c = (t - 1) // 2 - W_LAG
                return (c, 1) if 0 <= c < nchunk else None

            # prefetch
            for t in range(PF):
                j = job_at(t)
                if j is not None and j[1] == 0:
                    load(j[0])

            T = 2 * nchunk + 2 * W_LAG + 1
            for t in range(T + 5):
                jl = job_at(t + PF)
                if jl is not None and jl[1] == 0:
                    load(jl[0])
                # PE stages; every dependency is >=1 tick old
                j1 = job_at(t - 2)
                if j1 is not None:
                    s3_qsw(*j1)
                j2 = job_at(t - 3)
                if j2 is not None:
                    s5_a1mix(*j2)
                j3 = job_at(t - 4)
                if j3 is not None:
                    s7_a2(*j3)
                j0 = job_at(t)
                if j0 is not None:
                    s1_m1(*j0)
                # evacs: deps produced early this tick go first per engine
                if j2 is not None:
                    s6_relu(*j2)      # ACT: dep S5 (PE op #2)
                if j1 is not None:
                    s4_copy(*j1)      # DVE: dep S3 (PE op #1)
                if j3 is not None:
                    s8_res(*j3)       # DVE: dep S7 (PE op #3)
                if j0 is not None:
                    s2_relu(*j0)      # ACT: dep S1 (PE op #4, tick end)

    nc.compile()
    _nc_cache[cols] = nc
    return nc


def host_prep(module_states, connection_matrix, module_weights,
              mW1, mb1, mW2, mb2, aW1, ab1, aW2, ab2, ncores=NCORES):
    ms = np.asarray(module_states, np.float32)
    C = np.asarray(connection_matrix, np.float32)
    w = np.asarray(module_weights, np.float32)
    mW1 = np.asarray(mW1, np.float32)
    mb1 = np.asarray(mb1, np.float32)
    mW2 = np.asarray(mW2, np.float32)
    mb2 = np.asarray(mb2, np.float32)
    aW1 = np.asarray(aW1, np.float32)
    ab1 = np.asarray(ab1, np.float32)
    aW2 = np.asarray(aW2, np.float32)
    ab2 = np.asarray(ab2, np.float32)

    B = ms.shape[0]
    bsh = B // ncores
    cols = bsh * M
    nchunk = cols // CHUNK

    qb = mb2 @ aW1[D:, :]                                    # [128]
    consts = {
        "wm1": mW1.astype(BF16),
        "wq": (mW2 @ aW1[D:, :]).astype(BF16),
        "wa1t": np.ascontiguousarray(aW1[:D, :]).astype(BF16),
        "wa2": aW2.astype(BF16),
        "mb10": np.ascontiguousarray(mb1.reshape(D, 1)),
        # step-1 operand is c1b = c1 - ab2; fold ab2 through the linears
        "mb11": np.ascontiguousarray((mb1 + mW1.T @ ab2).reshape(D, 1)),
        "qbt": np.ascontiguousarray(np.tile(qb, (D, CHUNK // D))),
        "ab10": np.ascontiguousarray(ab1.reshape(D, 1)),
        "ab11": np.ascontiguousarray((ab1 + aW1[:D].T @ ab2).reshape(D, 1)),
    }

    kron16 = np.kron(np.eye(16, dtype=np.float32), C.T)      # [128, 128]
    kron_row = np.tile(kron16, (1, NBLK))                    # [128, CHUNK]

    in_maps = []
    for k in range(ncores):
        shard = ms[k * bsh : (k + 1) * bsh]
        xT = shard.reshape(cols, D).T                        # [128, cols]
        xb = np.ascontiguousarray(
            xT.reshape(D, nchunk, CHUNK).transpose(1, 0, 2)
        ).astype(BF16)
        wflat = w[k * bsh : (k + 1) * bsh].reshape(nchunk, CHUNK)
        wck = (kron_row[None, :, :] * wflat[:, None, :]).astype(BF16)
        in_maps.append({"xb": xb, "wck": wck, **consts})
    return cols, in_maps


def gather_out(results, ab2, ncores=NCORES):
    """Device out = bf16(x) + up0 + up1 (ab2 folded); host adds 2*ab2."""
    ab2 = np.asarray(ab2, np.float32)
    outs = []
    for k in range(ncores):
        o = np.asarray(results[k]["out"]).astype(np.float32)
        nchunk = o.shape[0]
        cols = nchunk * CHUNK
        bsh = cols // M
        oT = o.transpose(1, 0, 2).reshape(D, cols)
        outs.append(oT.T.reshape(bsh, M, D))
    out = np.concatenate(outs, 0)
    # reference adds ab2 each step; c1b excludes it (folded through the
    # step-1 linears), so the host restores both here.
    out += 2.0 * ab2[None, None, :]
    return out.astype(np.float32)


def _run(inputs, trace=False):
    from concourse.bass_utils import run_bass_kernel_spmd

    cols, in_maps = host_prep(**inputs)
    nc = build_nc(cols)
    res = run_bass_kernel_spmd(nc, in_maps, list(range(NCORES)), trace=trace)
    out = gather_out(res.results, inputs["ab2"])
    return out, res


def kernel(**inputs):
    out, _ = _run(inputs, trace=False)
    return out
